# revision 2
# baseline (speedup 1.0000x reference)
"""Trainium2 Bass kernel for BondLengthConstraintEnergy.

Contract: kernel(**inputs) takes FULL unsharded inputs (as produced by the
problem's setup_inputs) and returns the FULL output [B, NCH, NRES, n_alt].

Strategy
--------
The input layout produced by setup_inputs is canonical: atom i corresponds to
(b, ch, r, a) = unravel(i) over (32, 8, 8192, 3), so the (b,ch,r,atom)->row
lookup table is exactly arange, every peptide bond (b,ch,r)->(b,ch,r+1) is
present, and the per-residue-type mean/std tables have identical rows.  Under
those conditions (verified on the host each call) the whole computation
collapses to a pure streaming stencil over coords:

  per bond r (residue r, r+1 in the same chain):
    b = C_r - CA_r          (v_cac_c)
    w = N_{r+1} - C_r       (v_cn)
    a = CA_{r+1} - N_{r+1}  (v_nca_n)
    ang1 = angle(w, a), ang2 = angle(b, -w), len = |w|
    lp_i  = min(d_i^2 / (2 var_i), -ln(EPS) - ln(sqrt(2 pi var_i)))
    out[b,ch,r,0] = (lp0+lp1+lp2) * (1 - tanh(-weight))

Angles are computed without any acos on device via
    theta = pi/2 - atan(dot / sqrt(|u|^2 |v|^2 - dot^2))
which is exact for theta in (0, pi) and numerically great in the region where
the gaussian is not clipped.

Sharding: data-parallel over batch, 4 structures per core, no communication.
Each core streams 9.4 MB of coords and writes 1 MB of energies.

If the host-side structure checks fail (inputs are not canonical), we fall
back to a faithful numpy implementation of the reference.
"""

import os
import sys

import numpy as np

for _p in ("/opt/trn_rl_repo",):
    if os.path.isdir(_p) and _p not in sys.path:
        sys.path.insert(0, _p)

# ---------------------------------------------------------------- constants
B, NCH, NRES, APR = 32, 8, 8192, 3
N_ATOMS = B * NCH * NRES * APR
NCORES = 8
B_PER_CORE = B // NCORES
RES_PER_CORE = B_PER_CORE * NCH * NRES          # 262144
ATOMS_PER_CORE = RES_PER_CORE * APR
P = 128                                          # SBUF partitions
W = 256                                          # bonds per partition per tile
RES_PER_PART = RES_PER_CORE // P                 # 2048
NT = RES_PER_PART // W                           # 8 tiles per core
EPS = 1e-8
NEG_LOG_EPS = 18.420680743952367                 # -ln(1e-8)
R2D = 180.0 / np.pi
TINY = 1e-38

# benign pad residue (N=(0,0,0), CA=(1,0,0), C=(2,0,0)) keeps the one
# out-of-range halo bond finite; its output is overwritten on the host.
_PAD_RESIDUE = np.array([0, 0, 0, 1, 0, 0, 2, 0, 0], dtype=np.float32)

_PROGRAM = None


# ---------------------------------------------------------------- device IR
def _build_program(reps=1, cfg=None):
    """Build + compile the per-core Bass/Tile program (identical on all cores).

    reps>1 wraps the whole body in a device-side loop — used only by the
    timing harness to amplify kernel time over dispatch/transfer noise.
    """
    import concourse.bacc as bacc
    import concourse.bass as bass
    import concourse.mybir as mybir
    import concourse.tile as tile

    import bass_rust

    cfg = dict(cfg or {})
    W = cfg.get("W", 256)
    tiles = cfg.get("tiles")
    if tiles is None:
        tiles = [W] * (RES_PER_PART // W)
    assert sum(tiles) == RES_PER_PART
    offs = [0]
    for w_ in tiles:
        offs.append(offs[-1] + w_)
    NT = len(tiles)
    xbufs = cfg.get("xbufs", 2)
    midbufs = cfg.get("midbufs", 2)
    bigbufs = cfg.get("bigbufs", 2)
    ph2bufs = cfg.get("ph2bufs", 3)

    dt = mybir.dt
    Alu = mybir.AluOpType
    Act = mybir.ActivationFunctionType

    nc = bacc.Bacc(
        "TRN2",
        target_bir_lowering=False,
        debug=False,
        enable_asserts=False,
        num_devices=NCORES,
    )

    xin = nc.dram_tensor("xin", [(RES_PER_CORE + 1) * 9], dt.float32,
                         kind="ExternalInput")
    cst = nc.dram_tensor("consts", [P, 16], dt.float32, kind="ExternalInput")
    out = nc.dram_tensor("out", [RES_PER_CORE], dt.float32,
                         kind="ExternalOutput")



    with tile.TileContext(nc) as tc:
        with (
            tc.tile_pool(name="cpool", bufs=1) as cpool,
            tc.tile_pool(name="xpool", bufs=xbufs) as xpool,
            tc.tile_pool(name="dpool", bufs=bigbufs) as dpool,
            tc.tile_pool(name="spool", bufs=bigbufs) as spool,
            tc.tile_pool(name="ppool", bufs=bigbufs) as ppool,
            tc.tile_pool(name="mid", bufs=midbufs) as mid,
            tc.tile_pool(name="xph", bufs=NT) as xph,     # crosses phase bound
            tc.tile_pool(name="ph2", bufs=ph2bufs) as ph2,
        ):
            ctile = cpool.tile([P, 16], dt.float32, tag="c")
            nc.sync.dma_start(ctile[:, :], cst.ap())
            c_bias1 = ctile[:, 0:1]
            c_bias2 = ctile[:, 1:2]
            c_k1 = ctile[:, 2:3]
            c_c1 = ctile[:, 3:4]
            c_k2 = ctile[:, 4:5]
            c_c2 = ctile[:, 5:6]
            c_nm0 = ctile[:, 6:7]
            c_k0 = ctile[:, 7:8]
            c_c0 = ctile[:, 8:9]

            def _body():
                ratios = []
                d0sqs = []
                ph1_act = []

                def emit_ph2(t, W, ratio, d0sq):

                    h = ph2.tile([P, 2 * W], dt.float32, tag="h")
                    h_inst = nc.scalar.activation(h[:, :], ratio[:, :], Act.Arctan)
                    if cfg.get('fence', False):
                        for a in ph1_act:
                            bass_rust.add_dep_helper(
                                h_inst.ins, a.ins,
                                reason="act set fence: all sqrt-set before arctan")
                    hv = h[:, :].rearrange("p (w t) -> p w t", t=2)
                    # ang1 (odd slots):  d1 = -R2D*(h1 - (90-m1)/R2D)
                    # ang2 (even slots): d2 =  R2D*(h2 + (90-m2)/R2D)
                    sq1 = ph2.tile([P, W], dt.float32, tag="sq1")
                    nc.scalar.activation(sq1[:, :], hv[:, :, 1], Act.Square,
                                         bias=c_bias1)
                    sq2 = ph2.tile([P, W], dt.float32, tag="sq2")
                    nc.scalar.activation(sq2[:, :], hv[:, :, 0], Act.Square,
                                         bias=c_bias2)

                    lp0 = ph2.tile([P, W], dt.float32, tag="lp0")
                    nc.vector.tensor_scalar(lp0[:, :], d0sq[:, :], c_k0, c_c0,
                                            op0=Alu.mult, op1=Alu.min)
                    lp1 = ph2.tile([P, W], dt.float32, tag="lp1")
                    nc.vector.tensor_scalar(lp1[:, :], sq1[:, :], c_k1, c_c1,
                                            op0=Alu.mult, op1=Alu.min)
                    lp2 = ph2.tile([P, W], dt.float32, tag="lp2")
                    nc.vector.tensor_scalar(lp2[:, :], sq2[:, :], c_k2, c_c2,
                                            op0=Alu.mult, op1=Alu.min)

                    sum_eng = nc.gpsimd if cfg.get("sum_eng", "gpsimd") == "gpsimd" else nc.vector
                    s01 = ph2.tile([P, W], dt.float32, tag="s01")
                    sum_eng.tensor_tensor(s01[:, :], lp0[:, :], lp1[:, :],
                                          op=Alu.add)
                    val = ph2.tile([P, W], dt.float32, tag="val")
                    sum_eng.tensor_tensor(val[:, :], s01[:, :], lp2[:, :],
                                          op=Alu.add)

                    dst = bass.AP(out, P * offs[t], [[W, P], [1, W]])
                    nc.sync.dma_start(dst, val[:, :])

                # ---------------- phase 1: everything up to atan inputs --------
                for t in range(NT):
                    W = tiles[t]
                    FW = 9 * W
                    XW = 9 * (W + 1)
                    base = P * offs[t]
                    x = xpool.tile([P, XW], dt.float32, tag="x")
                    if cfg.get('dma_split', 1) == 2:
                        h1 = XW // 2
                        nc.sync.dma_start(
                            x[:, 0:h1],
                            bass.AP(xin, base * 9, [[FW, P], [1, h1]]))
                        nc.sync.dma_start(
                            x[:, h1:XW],
                            bass.AP(xin, base * 9 + h1, [[FW, P], [1, XW - h1]]))
                    else:
                        src = bass.AP(xin, base * 9, [[FW, P], [1, XW]])
                        nc.sync.dma_start(x[:, :], src)

                    # D[i] = X[i+6] - X[i+3]; per group j (bond j):
                    #   D[9j+0..2]=v_cac, D[9j+3..5]=v_cn, D[9j+6..8]=v_nca
                    d = dpool.tile([P, FW], dt.float32, tag="d")
                    if cfg.get('d_eng', 'gpsimd') == 'dve' or t < cfg.get('d_dve_tiles', 1):
                        nc.vector.tensor_sub(d[:, :], x[:, 6:6 + FW], x[:, 3:3 + FW])
                    else:
                        k = int(FW * cfg.get('d_split', 1.0))
                        if cfg.get('d_pool2', False) and k == FW:
                            h = FW // 2
                            nc.gpsimd.tensor_sub(d[:, 0:h], x[:, 6:6 + h],
                                                 x[:, 3:3 + h])
                            nc.gpsimd.tensor_sub(d[:, h:FW], x[:, 6 + h:6 + FW],
                                                 x[:, 3 + h:3 + FW])
                        else:
                            if k > 0:
                                nc.gpsimd.tensor_sub(d[:, 0:k], x[:, 6:6 + k],
                                                     x[:, 3:3 + k])
                            if k < FW:
                                nc.vector.tensor_sub(d[:, k:FW], x[:, 6 + k:6 + FW],
                                                     x[:, 3 + k:3 + FW])

                    # squares of all components (scalar engine, sqrt-family set)
                    s = spool.tile([P, FW], dt.float32, tag="s")
                    ph1_act.append(nc.scalar.activation(s[:, :], d[:, :], Act.Square))

                    # P6[6j+m] = D[9j+m]*D[9j+m+3], m=0..5
                    #   m=0..2 -> v_cac.v_cn terms (dot2), m=3..5 -> v_cn.v_nca (dot1)
                    d3 = d[:, :].rearrange("p (w k) -> p w k", k=9)
                    p6 = ppool.tile([P, 6 * W], dt.float32, tag="p6")
                    p6v = p6[:, :].rearrange("p (w k) -> p w k", k=6)
                    nc.vector.tensor_tensor(p6v, d3[:, :, 0:6], d3[:, :, 3:9],
                                            op=Alu.mult)

                    # windowed 3-sums of squares: R2[j] = (nc2, na2, nb2)
                    sv = s[:, :].rearrange("p (w t k) -> p w t k", t=3, k=3)
                    r2 = mid.tile([P, 3 * W], dt.float32, tag="r2")
                    r2v = r2[:, :].rearrange("p (w t) -> p w t", t=3)
                    nc.vector.tensor_tensor(r2v, sv[:, :, :, 0], sv[:, :, :, 1],
                                            op=Alu.add)
                    nc.vector.tensor_tensor(r2v, r2v, sv[:, :, :, 2], op=Alu.add)

                    # dots (gpsimd): DOTS[j] = (dot2, dot1)
                    pv = p6[:, :].rearrange("p (w t k) -> p w t k", t=2, k=3)
                    dots = mid.tile([P, 2 * W], dt.float32, tag="dots")
                    dotsv = dots[:, :].rearrange("p (w t) -> p w t", t=2)
                    dots_eng = nc.gpsimd if cfg.get('dots_eng', 'dve') == 'gpsimd' else nc.vector
                    dots_eng.tensor_tensor(dotsv, pv[:, :, :, 0], pv[:, :, :, 1],
                                            op=Alu.add)
                    dots_eng.tensor_tensor(dotsv, dotsv, pv[:, :, :, 2],
                                            op=Alu.add)

                    # q interleaved to match DOTS: (q2, q1) = (nc2*na2, na2*nb2)
                    r2t = r2[:, :].rearrange("p (w t) -> p w t", t=3)
                    q = mid.tile([P, 2 * W], dt.float32, tag="q")
                    qv = q[:, :].rearrange("p (w t) -> p w t", t=2)
                    q_eng = nc.gpsimd if cfg.get("q_eng", "gpsimd") == "gpsimd" else nc.vector
                    if cfg.get("q_merge", False):
                        q_eng.tensor_tensor(qv, r2t[:, :, 0:2], r2t[:, :, 1:3],
                                            op=Alu.mult)
                    else:
                        q_eng.tensor_tensor(qv[:, :, 0:1], r2t[:, :, 0:1],
                                            r2t[:, :, 1:2], op=Alu.mult)
                        q_eng.tensor_tensor(qv[:, :, 1:2], r2t[:, :, 1:2],
                                            r2t[:, :, 2:3], op=Alu.mult)

                    # v = q - dot^2, clamped positive
                    dsq = mid.tile([P, 2 * W], dt.float32, tag="dsq")
                    ph1_act.append(nc.scalar.activation(dsq[:, :], dots[:, :], Act.Square))
                    v = mid.tile([P, 2 * W], dt.float32, tag="v")
                    nc.vector.scalar_tensor_tensor(v[:, :], dsq[:, :], -1.0,
                                                   q[:, :], op0=Alu.mult,
                                                   op1=Alu.add)
                    vce = cfg.get('vc_eng', 'act')
                    if vce == 'act':
                        vc = mid.tile([P, 2 * W], dt.float32, tag="vc")
                        ph1_act.append(nc.scalar.activation(vc[:, :], v[:, :],
                                                            Act.Relu))
                        v = vc
                    elif vce == 'gpsimd':
                        nc.gpsimd.tensor_scalar(v[:, :], v[:, :], TINY, None,
                                                op0=Alu.max)
                    else:
                        nc.vector.tensor_scalar(v[:, :], v[:, :], TINY, None,
                                                op0=Alu.max)

                    # sqrt(v); na = sqrt(na2); d0sq = (na - mean_len)^2
                    sq = mid.tile([P, 2 * W], dt.float32, tag="sq")
                    ph1_act.append(nc.scalar.activation(sq[:, :], v[:, :], Act.Sqrt))
                    na = mid.tile([P, W], dt.float32, tag="na")
                    ph1_act.append(nc.scalar.activation(na[:, :], r2t[:, :, 1], Act.Sqrt))
                    d0sq = xph.tile([P, W], dt.float32, tag="d0sq")
                    ph1_act.append(nc.scalar.activation(d0sq[:, :], na[:, :], Act.Square,
                                         bias=c_nm0))

                    # ratio = dot / sqrt(v)   (= cot(theta)), clamped to the
                    # scalar engine's arctan domain [-pi/2, pi/2].  Saturation
                    # maps theta into [32.5, 147.5] deg, far inside the gaussian
                    # clip zone on both sides, so clamped bonds stay exact.
                    rraw = mid.tile([P, 2 * W], dt.float32, tag="rraw")
                    if cfg.get('div', False):
                        nc.vector.tensor_tensor(rraw[:, :], dots[:, :], sq[:, :],
                                                op=Alu.divide)
                    else:
                        rv = mid.tile([P, 2 * W], dt.float32, tag="rv")
                        nc.vector.reciprocal(rv[:, :], sq[:, :])
                        nc.vector.tensor_tensor(rraw[:, :], dots[:, :], rv[:, :],
                                                op=Alu.mult)
                    ratio = xph.tile([P, 2 * W], dt.float32, tag="ratio")
                    nc.vector.tensor_scalar(ratio[:, :], rraw[:, :], 1.5707,
                                            -1.5707, op0=Alu.min, op1=Alu.max)

                    ratios.append(ratio)
                    d0sqs.append(d0sq)
                    if cfg.get('interleave', False):
                        emit_ph2(t, W, ratio, d0sq)

                if not cfg.get('interleave', False):
                    for t in range(NT):
                        emit_ph2(t, tiles[t], ratios[t], d0sqs[t])

            if reps == 1:
                _body()
            else:
                with tc.For_i(0, reps, 1):
                    _body()

    nc.compile()
    return nc


def _get_program():
    global _PROGRAM
    if _PROGRAM is None:
        _PROGRAM = _build_program()
    return _PROGRAM


# ---------------------------------------------------------------- host side
def _make_consts(mean_row, std_row, weight0):
    m = np.asarray(mean_row, dtype=np.float64)
    s = np.asarray(std_row, dtype=np.float64)
    f = 1.0 - np.tanh(-float(weight0))
    var = s * s
    clip = NEG_LOG_EPS - 0.5 * np.log(2.0 * np.pi * var)
    c = np.zeros(16, dtype=np.float64)
    c[0] = (m[1] - 90.0) / R2D                    # bias1
    c[1] = (90.0 - m[2]) / R2D                    # bias2
    c[2] = (R2D * R2D) / (2.0 * var[1]) * f       # K1'
    c[3] = clip[1] * f                            # C1'
    c[4] = (R2D * R2D) / (2.0 * var[2]) * f       # K2'
    c[5] = clip[2] * f                            # C2'
    c[6] = -m[0]                                  # -mean_len
    c[7] = 1.0 / (2.0 * var[0]) * f               # K0'
    c[8] = clip[0] * f                            # C0'
    return np.tile(c.astype(np.float32), (P, 1))


def _is_canonical(ad, coords, mean, std):
    if ad.shape != (N_ATOMS, 5) or coords.shape != (N_ATOMS, 3):
        return False
    if mean.shape != (20, 3) or std.shape != (20, 3):
        return False
    if not (np.all(mean == mean[0:1]) and np.all(std == std[0:1])):
        return False
    if not np.all(std[0] > 0):
        return False
    a5 = ad.reshape(B, NCH, NRES, APR, 5)
    if not np.all(a5[..., 0] == np.arange(B, dtype=ad.dtype)[:, None, None, None]):
        return False
    if not np.all(a5[..., 1] == np.arange(NCH, dtype=ad.dtype)[:, None, None]):
        return False
    if not np.all(a5[..., 2] == np.arange(NRES, dtype=ad.dtype)[:, None]):
        return False
    if not np.all(a5[..., 4] == np.arange(APR, dtype=ad.dtype)):
        return False
    if not np.isfinite(coords).all() or np.abs(coords).max() >= 1e4:
        return False
    # all bond-geometry norms must clear the reference's EPS mask, so the
    # device kernel can skip mask arithmetic entirely
    r = coords.reshape(B, NCH, NRES, 9)
    w = r[:, :, 1:, 0:3] - r[:, :, :-1, 6:9]
    a = r[:, :, 1:, 3:6] - r[:, :, 1:, 0:3]
    bb = r[:, :, :-1, 6:9] - r[:, :, :-1, 3:6]
    mn = min(
        (w * w).sum(-1).min(),
        (a * a).sum(-1).min(),
        (bb * bb).sum(-1).min(),
    )
    return bool(mn > 1.1e-16)


def _make_in_maps(coords, mean, std, weight):
    consts = _make_consts(mean[0], std[0], weight[0])
    cflat = np.ascontiguousarray(coords.reshape(-1), dtype=np.float32)
    in_maps = []
    for c in range(NCORES):
        shard = np.empty(((RES_PER_CORE + 1) * 9,), dtype=np.float32)
        shard[:-9] = cflat[c * ATOMS_PER_CORE * 3:(c + 1) * ATOMS_PER_CORE * 3]
        shard[-9:] = _PAD_RESIDUE
        in_maps.append({"xin": shard, "consts": consts})
    return in_maps


def _run_fast(coords, mean, std, weight, n_alt):
    from concourse import bass_utils

    nc = _get_program()
    in_maps = _make_in_maps(coords, mean, std, weight)

    res = bass_utils.run_bass_kernel_spmd(nc, in_maps,
                                          core_ids=list(range(NCORES)))
    parts = [np.asarray(res.results[c]["out"], dtype=np.float32)
             for c in range(NCORES)]
    e = np.concatenate(parts).reshape(B, NCH, NRES)
    e[:, :, NRES - 1] = 0.0          # no bond out of the last residue
    full = np.zeros((B, NCH, NRES, n_alt), dtype=np.float32)
    full[..., 0] = e
    return full


# ------------------------------------------------------------ numpy fallback
def _fallback(ad, coords, alternatives, weight, mean, std):
    """Faithful numpy port of the jax reference (incl. OOB drop/clamp)."""
    n_alt = alternatives.shape[-1]
    batch, chain, resnum = ad[:, 0], ad[:, 1], ad[:, 2]
    resname, at_name = ad[:, 3], ad[:, 4]
    n = ad.shape[0]

    table = np.full((B, NCH, NRES, APR), -1, dtype=np.int32)
    ok = ((batch >= 0) & (batch < B) & (chain >= 0) & (chain < NCH)
          & (resnum >= 0) & (resnum < NRES) & (at_name >= 0) & (at_name < APR))
    idx = np.arange(n, dtype=np.int32)
    table[batch[ok], chain[ok], resnum[ok], at_name[ok]] = idx[ok]

    c_idx = table[:, :, :-1, 2].reshape(-1)
    n_idx = table[:, :, 1:, 0].reshape(-1)
    cac_idx = table[:, :, :-1, 1].reshape(-1)
    can_idx = table[:, :, 1:, 1].reshape(-1)
    valid_idx = (c_idx >= 0) & (n_idx >= 0) & (cac_idx >= 0) & (can_idx >= 0)
    safe = lambda i: np.where(i >= 0, i, 0)

    co = coords.astype(np.float32)
    c_xyz = co[safe(c_idx)]
    n_xyz = co[safe(n_idx)]
    cac_xyz = co[safe(cac_idx)]
    can_xyz = co[safe(can_idx)]

    v_cn = n_xyz - c_xyz
    v_nca = can_xyz - n_xyz
    v_cac = c_xyz - cac_xyz

    def ang_deg(a, b):
        na = np.sqrt((a * a).sum(-1))
        nb = np.sqrt((b * b).sum(-1))
        mask = (na > EPS) & (nb > EPS)
        cos = np.clip((a * b).sum(-1) / (na * nb + EPS), -1.0, 1.0)
        return np.degrees(np.arccos(cos)).astype(np.float32), mask

    ang1, m1 = ang_deg(v_cn, v_nca)
    ang2, m2 = ang_deg(v_cac, -v_cn)
    bond_len = np.sqrt((v_cn * v_cn).sum(-1))
    valid = valid_idx & m1 & m2

    geom = np.stack([bond_len, ang1, ang2], axis=-1)
    seq = np.clip(resname[safe(c_idx)], 0, 19)
    var = (std.astype(np.float32)[seq]) ** 2
    denom = np.sqrt(2.0 * np.pi * var).astype(np.float32)
    num = np.exp(-((geom - mean.astype(np.float32)[seq]) ** 2) / (2.0 * var))
    log_prob = -(np.log(np.clip(num / denom, EPS, None)) + np.log(denom))
    scores = log_prob.sum(-1)

    f = np.float32(1.0 - np.tanh(-np.float32(weight[0])))
    val = np.where(valid, scores * f, 0.0).astype(np.float32)

    b_c = batch[safe(c_idx)]
    ch_c = chain[safe(c_idx)]
    r_c = resnum[safe(c_idx)]
    resi = np.zeros((B, NCH, NRES, n_alt), dtype=np.float32)
    ok2 = ((b_c >= 0) & (b_c < B) & (ch_c >= 0) & (ch_c < NCH)
           & (r_c >= 0) & (r_c < NRES))
    resi[b_c[ok2], ch_c[ok2], r_c[ok2], 0] = val[ok2]
    return resi


# ----------------------------------------------------------------- entry
def kernel(atom_description, coords, alternatives, weight, mean, std):
    ad = np.asarray(atom_description)
    co = np.asarray(coords, dtype=np.float32)
    al = np.asarray(alternatives)
    wt = np.asarray(weight, dtype=np.float32)
    mn = np.asarray(mean, dtype=np.float32)
    sd = np.asarray(std, dtype=np.float32)

    if _is_canonical(ad, co, mn, sd):
        return _run_fast(co, mn, sd, wt, al.shape[-1])
    return _fallback(ad, co, al, wt, mn, sd)



# revision 12
# speedup vs baseline: 1.1747x; 1.1747x over previous
"""Trainium2 Bass kernel for BondLengthConstraintEnergy.

Contract: kernel(**inputs) takes FULL unsharded inputs (as produced by the
problem's setup_inputs) and returns the FULL output [B, NCH, NRES, n_alt].

Strategy
--------
The input layout produced by setup_inputs is canonical: atom i corresponds to
(b, ch, r, a) = unravel(i) over (32, 8, 8192, 3), so the (b,ch,r,atom)->row
lookup table is exactly arange, every peptide bond (b,ch,r)->(b,ch,r+1) is
present, and the per-residue-type mean/std tables have identical rows.  Under
those conditions (verified on the host each call) the whole computation
collapses to a pure streaming stencil over coords:

  per bond r (residue r, r+1 in the same chain):
    b = C_r - CA_r          (v_cac_c)
    w = N_{r+1} - C_r       (v_cn)
    a = CA_{r+1} - N_{r+1}  (v_nca_n)
    ang1 = angle(w, a), ang2 = angle(b, -w), len = |w|
    lp_i  = min(d_i^2 / (2 var_i), -ln(EPS) - ln(sqrt(2 pi var_i)))
    out[b,ch,r,0] = (lp0+lp1+lp2) * (1 - tanh(-weight))

Angles are computed without any acos on device via
    theta = pi/2 - atan(dot / sqrt(|u|^2 |v|^2 - dot^2))
which is exact for theta in (0, pi) and numerically great in the region where
the gaussian is not clipped.

Sharding: data-parallel over batch, 4 structures per core, no communication.
Each core streams 9.4 MB of coords and writes 1 MB of energies.

If the host-side structure checks fail (inputs are not canonical), we fall
back to a faithful numpy implementation of the reference.
"""

import os
import sys

import numpy as np

for _p in ("/opt/trn_rl_repo",):
    if os.path.isdir(_p) and _p not in sys.path:
        sys.path.insert(0, _p)

# ---------------------------------------------------------------- constants
B, NCH, NRES, APR = 32, 8, 8192, 3
N_ATOMS = B * NCH * NRES * APR
NCORES = 8
B_PER_CORE = B // NCORES
RES_PER_CORE = B_PER_CORE * NCH * NRES          # 262144
ATOMS_PER_CORE = RES_PER_CORE * APR
P = 128                                          # SBUF partitions
W = 256                                          # bonds per partition per tile
RES_PER_PART = RES_PER_CORE // P                 # 2048
NT = RES_PER_PART // W                           # 8 tiles per core
EPS = 1e-8
NEG_LOG_EPS = 18.420680743952367                 # -ln(1e-8)
R2D = 180.0 / np.pi
TINY = 1e-38

# benign pad residue (N=(0,0,0), CA=(1,0,0), C=(2,0,0)) keeps the one
# out-of-range halo bond finite; its output is overwritten on the host.
_PAD_RESIDUE = np.array([0, 0, 0, 1, 0, 0, 2, 0, 0], dtype=np.float32)

_PROGRAM = None
_PROGRAM_V3 = None

# two benign pad diffs appended after the last real atom diff so the final
# bond's (w, a) reads stay in range; its output is host-overwritten anyway.
_PAD_DIFFS = np.array([[1.0, 0.0, 0.0], [1.0, 0.0, 0.0]], dtype=np.float32)


# ---------------------------------------------------------------- device IR
def _build_program(reps=1, cfg=None):
    """Build + compile the per-core Bass/Tile program (identical on all cores).

    reps>1 wraps the whole body in a device-side loop — used only by the
    timing harness to amplify kernel time over dispatch/transfer noise.
    """
    import concourse.bacc as bacc
    import concourse.bass as bass
    import concourse.mybir as mybir
    import concourse.tile as tile

    import bass_rust

    cfg = dict(cfg or {})
    W = cfg.get("W", 256)
    tiles = cfg.get("tiles")
    if tiles is None:
        tiles = [W] * (RES_PER_PART // W)
    assert sum(tiles) == RES_PER_PART
    offs = [0]
    for w_ in tiles:
        offs.append(offs[-1] + w_)
    NT = len(tiles)
    xbufs = cfg.get("xbufs", 2)
    midbufs = cfg.get("midbufs", 2)
    bigbufs = cfg.get("bigbufs", 2)
    ph2bufs = cfg.get("ph2bufs", 3)

    dt = mybir.dt
    Alu = mybir.AluOpType
    Act = mybir.ActivationFunctionType

    nc = bacc.Bacc(
        "TRN2",
        target_bir_lowering=False,
        debug=False,
        enable_asserts=False,
        num_devices=NCORES,
    )

    xin = nc.dram_tensor("xin", [(RES_PER_CORE + 1) * 9], dt.float32,
                         kind="ExternalInput")
    cst = nc.dram_tensor("consts", [P, 16], dt.float32, kind="ExternalInput")
    out = nc.dram_tensor("out", [RES_PER_CORE], dt.float32,
                         kind="ExternalOutput")



    with tile.TileContext(nc) as tc:
        with (
            tc.tile_pool(name="cpool", bufs=1) as cpool,
            tc.tile_pool(name="xpool", bufs=xbufs) as xpool,
            tc.tile_pool(name="dpool", bufs=bigbufs) as dpool,
            tc.tile_pool(name="spool", bufs=bigbufs) as spool,
            tc.tile_pool(name="ppool", bufs=bigbufs) as ppool,
            tc.tile_pool(name="mid", bufs=midbufs) as mid,
            tc.tile_pool(name="xph", bufs=NT) as xph,     # crosses phase bound
            tc.tile_pool(name="ph2", bufs=ph2bufs) as ph2,
        ):
            ctile = cpool.tile([P, 16], dt.float32, tag="c")
            nc.sync.dma_start(ctile[:, :], cst.ap())
            c_bias1 = ctile[:, 0:1]
            c_bias2 = ctile[:, 1:2]
            c_k1 = ctile[:, 2:3]
            c_c1 = ctile[:, 3:4]
            c_k2 = ctile[:, 4:5]
            c_c2 = ctile[:, 5:6]
            c_nm0 = ctile[:, 6:7]
            c_k0 = ctile[:, 7:8]
            c_c0 = ctile[:, 8:9]

            def _body():
                ratios = []
                d0sqs = []
                ph1_act = []

                def emit_ph2(t, W, ratio, d0sq):

                    h = ph2.tile([P, 2 * W], dt.float32, tag="h")
                    h_inst = nc.scalar.activation(h[:, :], ratio[:, :], Act.Arctan)
                    if cfg.get('fence', False):
                        for a in ph1_act:
                            bass_rust.add_dep_helper(
                                h_inst.ins, a.ins,
                                reason="act set fence: all sqrt-set before arctan")
                    hv = h[:, :].rearrange("p (w t) -> p w t", t=2)
                    # ang1 (odd slots):  d1 = -R2D*(h1 - (90-m1)/R2D)
                    # ang2 (even slots): d2 =  R2D*(h2 + (90-m2)/R2D)
                    sq1 = ph2.tile([P, W], dt.float32, tag="sq1")
                    nc.scalar.activation(sq1[:, :], hv[:, :, 1], Act.Square,
                                         bias=c_bias1)
                    sq2 = ph2.tile([P, W], dt.float32, tag="sq2")
                    nc.scalar.activation(sq2[:, :], hv[:, :, 0], Act.Square,
                                         bias=c_bias2)

                    lp0 = ph2.tile([P, W], dt.float32, tag="lp0")
                    nc.vector.tensor_scalar(lp0[:, :], d0sq[:, :], c_k0, c_c0,
                                            op0=Alu.mult, op1=Alu.min)
                    lp1 = ph2.tile([P, W], dt.float32, tag="lp1")
                    nc.vector.tensor_scalar(lp1[:, :], sq1[:, :], c_k1, c_c1,
                                            op0=Alu.mult, op1=Alu.min)
                    lp2 = ph2.tile([P, W], dt.float32, tag="lp2")
                    nc.vector.tensor_scalar(lp2[:, :], sq2[:, :], c_k2, c_c2,
                                            op0=Alu.mult, op1=Alu.min)

                    sum_eng = nc.gpsimd if cfg.get("sum_eng", "gpsimd") == "gpsimd" else nc.vector
                    s01 = ph2.tile([P, W], dt.float32, tag="s01")
                    sum_eng.tensor_tensor(s01[:, :], lp0[:, :], lp1[:, :],
                                          op=Alu.add)
                    val = ph2.tile([P, W], dt.float32, tag="val")
                    sum_eng.tensor_tensor(val[:, :], s01[:, :], lp2[:, :],
                                          op=Alu.add)

                    dst = bass.AP(out, P * offs[t], [[W, P], [1, W]])
                    nc.sync.dma_start(dst, val[:, :])

                # ---------------- phase 1: everything up to atan inputs --------
                for t in range(NT):
                    W = tiles[t]
                    FW = 9 * W
                    XW = 9 * (W + 1)
                    base = P * offs[t]
                    x = xpool.tile([P, XW], dt.float32, tag="x")
                    if cfg.get('dma_split', 1) == 2:
                        h1 = XW // 2
                        nc.sync.dma_start(
                            x[:, 0:h1],
                            bass.AP(xin, base * 9, [[FW, P], [1, h1]]))
                        nc.sync.dma_start(
                            x[:, h1:XW],
                            bass.AP(xin, base * 9 + h1, [[FW, P], [1, XW - h1]]))
                    else:
                        src = bass.AP(xin, base * 9, [[FW, P], [1, XW]])
                        nc.sync.dma_start(x[:, :], src)

                    # D[i] = X[i+6] - X[i+3]; per group j (bond j):
                    #   D[9j+0..2]=v_cac, D[9j+3..5]=v_cn, D[9j+6..8]=v_nca
                    d = dpool.tile([P, FW], dt.float32, tag="d")
                    if cfg.get('d_eng', 'gpsimd') == 'dve' or t < cfg.get('d_dve_tiles', 1):
                        nc.vector.tensor_sub(d[:, :], x[:, 6:6 + FW], x[:, 3:3 + FW])
                    else:
                        k = int(FW * cfg.get('d_split', 1.0))
                        if cfg.get('d_pool2', False) and k == FW:
                            h = FW // 2
                            nc.gpsimd.tensor_sub(d[:, 0:h], x[:, 6:6 + h],
                                                 x[:, 3:3 + h])
                            nc.gpsimd.tensor_sub(d[:, h:FW], x[:, 6 + h:6 + FW],
                                                 x[:, 3 + h:3 + FW])
                        else:
                            if k > 0:
                                nc.gpsimd.tensor_sub(d[:, 0:k], x[:, 6:6 + k],
                                                     x[:, 3:3 + k])
                            if k < FW:
                                nc.vector.tensor_sub(d[:, k:FW], x[:, 6 + k:6 + FW],
                                                     x[:, 3 + k:3 + FW])

                    # squares of all components (scalar engine, sqrt-family set)
                    s = spool.tile([P, FW], dt.float32, tag="s")
                    ph1_act.append(nc.scalar.activation(s[:, :], d[:, :], Act.Square))

                    # P6[6j+m] = D[9j+m]*D[9j+m+3], m=0..5
                    #   m=0..2 -> v_cac.v_cn terms (dot2), m=3..5 -> v_cn.v_nca (dot1)
                    d3 = d[:, :].rearrange("p (w k) -> p w k", k=9)
                    p6 = ppool.tile([P, 6 * W], dt.float32, tag="p6")
                    p6v = p6[:, :].rearrange("p (w k) -> p w k", k=6)
                    nc.vector.tensor_tensor(p6v, d3[:, :, 0:6], d3[:, :, 3:9],
                                            op=Alu.mult)

                    # windowed 3-sums of squares: R2[j] = (nc2, na2, nb2)
                    sv = s[:, :].rearrange("p (w t k) -> p w t k", t=3, k=3)
                    r2 = mid.tile([P, 3 * W], dt.float32, tag="r2")
                    r2v = r2[:, :].rearrange("p (w t) -> p w t", t=3)
                    nc.vector.tensor_tensor(r2v, sv[:, :, :, 0], sv[:, :, :, 1],
                                            op=Alu.add)
                    nc.vector.tensor_tensor(r2v, r2v, sv[:, :, :, 2], op=Alu.add)

                    # dots (gpsimd): DOTS[j] = (dot2, dot1)
                    pv = p6[:, :].rearrange("p (w t k) -> p w t k", t=2, k=3)
                    dots = mid.tile([P, 2 * W], dt.float32, tag="dots")
                    dotsv = dots[:, :].rearrange("p (w t) -> p w t", t=2)
                    dots_eng = nc.gpsimd if cfg.get('dots_eng', 'dve') == 'gpsimd' else nc.vector
                    dots_eng.tensor_tensor(dotsv, pv[:, :, :, 0], pv[:, :, :, 1],
                                            op=Alu.add)
                    dots_eng.tensor_tensor(dotsv, dotsv, pv[:, :, :, 2],
                                            op=Alu.add)

                    # q interleaved to match DOTS: (q2, q1) = (nc2*na2, na2*nb2)
                    r2t = r2[:, :].rearrange("p (w t) -> p w t", t=3)
                    q = mid.tile([P, 2 * W], dt.float32, tag="q")
                    qv = q[:, :].rearrange("p (w t) -> p w t", t=2)
                    q_eng = nc.gpsimd if cfg.get("q_eng", "gpsimd") == "gpsimd" else nc.vector
                    if cfg.get("q_merge", False):
                        q_eng.tensor_tensor(qv, r2t[:, :, 0:2], r2t[:, :, 1:3],
                                            op=Alu.mult)
                    else:
                        q_eng.tensor_tensor(qv[:, :, 0:1], r2t[:, :, 0:1],
                                            r2t[:, :, 1:2], op=Alu.mult)
                        q_eng.tensor_tensor(qv[:, :, 1:2], r2t[:, :, 1:2],
                                            r2t[:, :, 2:3], op=Alu.mult)

                    # v = q - dot^2, clamped positive
                    dsq = mid.tile([P, 2 * W], dt.float32, tag="dsq")
                    ph1_act.append(nc.scalar.activation(dsq[:, :], dots[:, :], Act.Square))
                    v = mid.tile([P, 2 * W], dt.float32, tag="v")
                    nc.vector.scalar_tensor_tensor(v[:, :], dsq[:, :], -1.0,
                                                   q[:, :], op0=Alu.mult,
                                                   op1=Alu.add)
                    vce = cfg.get('vc_eng', 'act')
                    if vce == 'act':
                        vc = mid.tile([P, 2 * W], dt.float32, tag="vc")
                        ph1_act.append(nc.scalar.activation(vc[:, :], v[:, :],
                                                            Act.Relu))
                        v = vc
                    elif vce == 'gpsimd':
                        nc.gpsimd.tensor_scalar(v[:, :], v[:, :], TINY, None,
                                                op0=Alu.max)
                    else:
                        nc.vector.tensor_scalar(v[:, :], v[:, :], TINY, None,
                                                op0=Alu.max)

                    # sqrt(v); na = sqrt(na2); d0sq = (na - mean_len)^2
                    sq = mid.tile([P, 2 * W], dt.float32, tag="sq")
                    ph1_act.append(nc.scalar.activation(sq[:, :], v[:, :], Act.Sqrt))
                    na = mid.tile([P, W], dt.float32, tag="na")
                    ph1_act.append(nc.scalar.activation(na[:, :], r2t[:, :, 1], Act.Sqrt))
                    d0sq = xph.tile([P, W], dt.float32, tag="d0sq")
                    ph1_act.append(nc.scalar.activation(d0sq[:, :], na[:, :], Act.Square,
                                         bias=c_nm0))

                    # ratio = dot / sqrt(v)   (= cot(theta)), clamped to the
                    # scalar engine's arctan domain [-pi/2, pi/2].  Saturation
                    # maps theta into [32.5, 147.5] deg, far inside the gaussian
                    # clip zone on both sides, so clamped bonds stay exact.
                    rraw = mid.tile([P, 2 * W], dt.float32, tag="rraw")
                    if cfg.get('div', False):
                        nc.vector.tensor_tensor(rraw[:, :], dots[:, :], sq[:, :],
                                                op=Alu.divide)
                    else:
                        rv = mid.tile([P, 2 * W], dt.float32, tag="rv")
                        nc.vector.reciprocal(rv[:, :], sq[:, :])
                        nc.vector.tensor_tensor(rraw[:, :], dots[:, :], rv[:, :],
                                                op=Alu.mult)
                    ratio = xph.tile([P, 2 * W], dt.float32, tag="ratio")
                    nc.vector.tensor_scalar(ratio[:, :], rraw[:, :], 1.5707,
                                            -1.5707, op0=Alu.min, op1=Alu.max)

                    ratios.append(ratio)
                    d0sqs.append(d0sq)
                    if cfg.get('interleave', False):
                        emit_ph2(t, W, ratio, d0sq)

                if not cfg.get('interleave', False):
                    for t in range(NT):
                        emit_ph2(t, tiles[t], ratios[t], d0sqs[t])

            if reps == 1:
                _body()
            else:
                with tc.For_i(0, reps, 1):
                    _body()

    nc.compile()
    return nc


def _build_program_v3(reps=1, cfg=None):
    """Planar-fp16 streaming kernel.

    Host ships per-bond difference vectors (b, w, a) as fp16 in a
    block-planar layout: for each block of W bonds, 9 contiguous planes of
    W values (b.x, b.y, b.z, w.x, w.y, w.z, a.x, a.y, a.z).  Every device
    op is then unit-stride, so fp16 DVE ops hit the 2x (TT) / 4x (TSp)
    perf modes.  The bond-length path is kept in fp32 (squares of the w
    planes redone on ACT) because lp0 is ~1400x more sensitive than the
    angle terms.

    ACT uses only {Square, Sqrt} in phase 1 and {Arctan, Square} in
    phase 2, with explicit deps forcing all phase-1 ACT before the first
    arctan: exactly 2 table loads.
    """
    import concourse.bacc as bacc
    import concourse.bass as bass
    import concourse.mybir as mybir
    import concourse.tile as tile

    import bass_rust

    cfg = dict(cfg or {})
    W = cfg.get("W", 512)
    NT = RES_PER_PART // W
    xbufs = cfg.get("xbufs", 2)
    midbufs = cfg.get("midbufs", 2)
    sa = cfg.get("sa", 2)          # how many of the 9 fp16 square planes ACT does
    use_div = cfg.get("div", False)
    fence = cfg.get("fence", True)

    dt = mybir.dt
    f16 = dt.float16
    f32 = dt.float32
    Alu = mybir.AluOpType
    Act = mybir.ActivationFunctionType

    nc = bacc.Bacc(
        "TRN2",
        target_bir_lowering=False,
        debug=False,
        enable_asserts=False,
        num_devices=NCORES,
    )

    xin = nc.dram_tensor("xin", [RES_PER_CORE * 9], f16, kind="ExternalInput")
    cst = nc.dram_tensor("consts", [P, 16], f32, kind="ExternalInput")
    out = nc.dram_tensor("out", [RES_PER_CORE], f16, kind="ExternalOutput")

    with tile.TileContext(nc) as tc:
        with (
            tc.tile_pool(name="cpool", bufs=1) as cpool,
            tc.tile_pool(name="xpool", bufs=xbufs) as xpool,
            tc.tile_pool(name="spool", bufs=midbufs) as spool,
            tc.tile_pool(name="mid", bufs=midbufs) as mid,
            tc.tile_pool(name="xph", bufs=NT) as xph,     # crosses phase bound
            tc.tile_pool(name="ph2", bufs=cfg.get("ph2bufs", 3)) as ph2,
        ):
            ctile = cpool.tile([P, 16], f32, tag="c")
            nc.sync.dma_start(ctile[:, :], cst.ap())
            c_bias1 = ctile[:, 0:1]
            c_bias2 = ctile[:, 1:2]
            c_k1 = ctile[:, 2:3]
            c_c1 = ctile[:, 3:4]
            c_k2 = ctile[:, 4:5]
            c_c2 = ctile[:, 5:6]
            c_nm0 = ctile[:, 6:7]
            c_k0 = ctile[:, 7:8]
            c_c0 = ctile[:, 8:9]

            def _body():
                ph1_act = []
                ph2_in = []

                # ---------------- phase 1 ----------------
                for t in range(NT):
                    FW = 9 * W
                    x = xpool.tile([P, FW], f16, tag="x")
                    nc.sync.dma_start(
                        x[:, :], bass.AP(xin, t * P * FW, [[FW, P], [1, FW]]))

                    # squares of all 9 planes (fp16), split ACT/DVE
                    s = spool.tile([P, FW], f16, tag="s")
                    c = sa * W
                    if c > 0:
                        ph1_act.append(nc.scalar.activation(
                            s[:, 0:c], x[:, 0:c], Act.Square))
                    if c < FW:
                        nc.vector.tensor_tensor(
                            s[:, c:FW], x[:, c:FW], x[:, c:FW], op=Alu.mult)

                    # w-plane squares again in fp32 for the length path
                    sw = mid.tile([P, 3 * W], f32, tag="sw")
                    ph1_act.append(nc.scalar.activation(
                        sw[:, :], x[:, 3 * W:6 * W], Act.Square))
                    na2 = mid.tile([P, W], f32, tag="na2")
                    na2_eng = nc.gpsimd if cfg.get("na2_eng", "gpsimd") == "gpsimd" else nc.vector
                    na2_eng.tensor_tensor(na2[:, :], sw[:, 0:W], sw[:, W:2 * W],
                                          op=Alu.add)
                    na2_eng.tensor_tensor(na2[:, :], na2[:, :], sw[:, 2 * W:3 * W],
                                          op=Alu.add)

                    # norms (fp16): r2 planes = (nc2, na2f, nb2)
                    sv = s[:, :].rearrange("p (g k w) -> p g k w", k=3, w=W)
                    r2 = mid.tile([P, 3 * W], f16, tag="r2")
                    r2v = r2[:, :].rearrange("p (g w) -> p g w", w=W)
                    nc.vector.tensor_tensor(r2v, sv[:, :, 0, :], sv[:, :, 1, :],
                                            op=Alu.add)
                    nc.vector.tensor_tensor(r2v, r2v, sv[:, :, 2, :], op=Alu.add)

                    # products (b.w | w.a) in one wide fp16 op
                    p6 = mid.tile([P, 6 * W], f16, tag="p6")
                    nc.vector.tensor_tensor(p6[:, :], x[:, 0:6 * W],
                                            x[:, 3 * W:9 * W], op=Alu.mult)

                    # dots = (dot2, dot1) via grouped 3-sums
                    pv = p6[:, :].rearrange("p (g k w) -> p g k w", k=3, w=W)
                    dots = mid.tile([P, 2 * W], f16, tag="dots")
                    dotsv = dots[:, :].rearrange("p (g w) -> p g w", w=W)
                    nc.vector.tensor_tensor(dotsv, pv[:, :, 0, :], pv[:, :, 1, :],
                                            op=Alu.add)
                    nc.vector.tensor_tensor(dotsv, dotsv, pv[:, :, 2, :],
                                            op=Alu.add)

                    # q = (nc2*na2f, na2f*nb2)  fp32
                    q = mid.tile([P, 2 * W], f32, tag="q")
                    q_eng = nc.gpsimd if cfg.get("q_eng", "gpsimd") == "gpsimd" else nc.vector
                    q_eng.tensor_tensor(q[:, :], r2[:, 0:2 * W],
                                        r2[:, W:3 * W], op=Alu.mult)

                    # dsq = dots^2 (fp32), v = q - dsq, clamp positive
                    dsq = mid.tile([P, 2 * W], f32, tag="dsq")
                    ph1_act.append(nc.scalar.activation(dsq[:, :], dots[:, :],
                                                        Act.Square))
                    v = mid.tile([P, 2 * W], f32, tag="v")
                    vsub_eng = nc.gpsimd if cfg.get("vsub_eng", "gpsimd") == "gpsimd" else nc.vector
                    vsub_eng.tensor_tensor(v[:, :], q[:, :], dsq[:, :],
                                           op=Alu.subtract)
                    vmax_eng = nc.gpsimd if cfg.get("vmax_eng", "dve") == "gpsimd" else nc.vector
                    vmax_eng.tensor_scalar(v[:, :], v[:, :], TINY, None,
                                           op0=Alu.max)

                    # sqv = sqrt(v) (fp16 out), ratio = dots / sqv, clamped
                    sqv = mid.tile([P, 2 * W], f16, tag="sqv")
                    ph1_act.append(nc.scalar.activation(sqv[:, :], v[:, :],
                                                        Act.Sqrt))
                    ratio = mid.tile([P, 2 * W], f16, tag="ratio")
                    if use_div:
                        nc.vector.tensor_tensor(ratio[:, :], dots[:, :],
                                                sqv[:, :], op=Alu.divide)
                    else:
                        rv = mid.tile([P, 2 * W], f16, tag="rv")
                        with nc.allow_low_precision(
                                reason="ratio is scale-invariant; fp16 "
                                       "suffices for the clipped angle terms"):
                            nc.vector.reciprocal(rv[:, :], sqv[:, :])
                        nc.vector.tensor_tensor(ratio[:, :], dots[:, :],
                                                rv[:, :], op=Alu.mult)
                    ratioc = xph.tile([P, 2 * W], f16, tag="ratioc")
                    nc.vector.tensor_scalar(ratioc[:, :], ratio[:, :], 1.5707,
                                            -1.5707, op0=Alu.min, op1=Alu.max)

                    # length path: na = sqrt(na2), d0sq = (na - m0)^2
                    na = mid.tile([P, W], f32, tag="na")
                    ph1_act.append(nc.scalar.activation(na[:, :], na2[:, :],
                                                        Act.Sqrt))
                    d0sq = xph.tile([P, W], f32, tag="d0sq")
                    ph1_act.append(nc.scalar.activation(d0sq[:, :], na[:, :],
                                                        Act.Square, bias=c_nm0))
                    ph2_in.append((ratioc, d0sq))

                # ---------------- phase 2 ----------------
                for t in range(NT):
                    ratioc, d0sq = ph2_in[t]
                    h = ph2.tile([P, 2 * W], f32, tag="h")
                    h_inst = nc.scalar.activation(h[:, :], ratioc[:, :],
                                                  Act.Arctan)
                    if fence:
                        for a in ph1_act:
                            bass_rust.add_dep_helper(
                                h_inst.ins, a.ins,
                                reason="act table fence: ph1 before arctan")
                    sq1 = ph2.tile([P, W], f32, tag="sq1")
                    nc.scalar.activation(sq1[:, :], h[:, W:2 * W], Act.Square,
                                         bias=c_bias1)
                    sq2 = ph2.tile([P, W], f32, tag="sq2")
                    nc.scalar.activation(sq2[:, :], h[:, 0:W], Act.Square,
                                         bias=c_bias2)

                    lp0 = ph2.tile([P, W], f32, tag="lp0")
                    nc.vector.tensor_scalar(lp0[:, :], d0sq[:, :], c_k0, c_c0,
                                            op0=Alu.mult, op1=Alu.min)
                    lp1 = ph2.tile([P, W], f32, tag="lp1")
                    nc.vector.tensor_scalar(lp1[:, :], sq1[:, :], c_k1, c_c1,
                                            op0=Alu.mult, op1=Alu.min)
                    lp2 = ph2.tile([P, W], f32, tag="lp2")
                    nc.vector.tensor_scalar(lp2[:, :], sq2[:, :], c_k2, c_c2,
                                            op0=Alu.mult, op1=Alu.min)

                    sum_eng = nc.gpsimd if cfg.get("sum_eng", "gpsimd") == "gpsimd" else nc.vector
                    s01 = ph2.tile([P, W], f32, tag="s01")
                    sum_eng.tensor_tensor(s01[:, :], lp0[:, :], lp1[:, :],
                                          op=Alu.add)
                    val = ph2.tile([P, W], f16, tag="val")
                    sum_eng.tensor_tensor(val[:, :], s01[:, :], lp2[:, :],
                                          op=Alu.add)

                    dst = bass.AP(out, t * P * W, [[W, P], [1, W]])
                    nc.sync.dma_start(dst, val[:, :])

            if reps == 1:
                _body()
            else:
                with tc.For_i(0, reps, 1):
                    _body()

    nc.compile()
    return nc


def _get_program():
    global _PROGRAM
    if _PROGRAM is None:
        _PROGRAM = _build_program()
    return _PROGRAM


def _get_program_v3():
    global _PROGRAM_V3
    if _PROGRAM_V3 is None:
        _PROGRAM_V3 = _build_program_v3()
    return _PROGRAM_V3


def _make_in_maps_v3(coords, mean, std, weight, W=512):
    """Host prep for the v3 kernel: consecutive-atom diffs, block-planar
    fp16 layout, one contiguous shard slice per core."""
    consts = _make_consts(mean[0], std[0], weight[0])
    X = np.asarray(coords, dtype=np.float32)
    D = np.concatenate([X[1:] - X[:-1], _PAD_DIFFS], axis=0)
    NB = N_ATOMS // 3                      # bonds incl. chain-end garbage
    b = D[1::3][:NB]
    w = D[2::3][:NB]
    a = D[3::3][:NB]
    arr = np.concatenate([b, w, a], axis=1)            # [NB, 9]
    blocks = arr.reshape(-1, W, 9).transpose(0, 2, 1)  # [NBLK, 9, W]
    flat = np.ascontiguousarray(blocks).astype(np.float16).reshape(NCORES, -1)
    return [{"xin": flat[c], "consts": consts} for c in range(NCORES)]


def _run_fast_v3(coords, mean, std, weight, n_alt):
    from concourse import bass_utils

    nc = _get_program_v3()
    in_maps = _make_in_maps_v3(coords, mean, std, weight)
    res = bass_utils.run_bass_kernel_spmd(nc, in_maps,
                                          core_ids=list(range(NCORES)))
    parts = [np.asarray(res.results[c]["out"]) for c in range(NCORES)]
    e = np.concatenate(parts).astype(np.float32).reshape(B, NCH, NRES)
    e[:, :, NRES - 1] = 0.0          # no bond out of the last residue
    full = np.zeros((B, NCH, NRES, n_alt), dtype=np.float32)
    full[..., 0] = e
    return full


# ---------------------------------------------------------------- host side
def _make_consts(mean_row, std_row, weight0):
    m = np.asarray(mean_row, dtype=np.float64)
    s = np.asarray(std_row, dtype=np.float64)
    f = 1.0 - np.tanh(-float(weight0))
    var = s * s
    clip = NEG_LOG_EPS - 0.5 * np.log(2.0 * np.pi * var)
    c = np.zeros(16, dtype=np.float64)
    c[0] = (m[1] - 90.0) / R2D                    # bias1
    c[1] = (90.0 - m[2]) / R2D                    # bias2
    c[2] = (R2D * R2D) / (2.0 * var[1]) * f       # K1'
    c[3] = clip[1] * f                            # C1'
    c[4] = (R2D * R2D) / (2.0 * var[2]) * f       # K2'
    c[5] = clip[2] * f                            # C2'
    c[6] = -m[0]                                  # -mean_len
    c[7] = 1.0 / (2.0 * var[0]) * f               # K0'
    c[8] = clip[0] * f                            # C0'
    return np.tile(c.astype(np.float32), (P, 1))


def _is_canonical(ad, coords, mean, std):
    if ad.shape != (N_ATOMS, 5) or coords.shape != (N_ATOMS, 3):
        return False
    if mean.shape != (20, 3) or std.shape != (20, 3):
        return False
    if not (np.all(mean == mean[0:1]) and np.all(std == std[0:1])):
        return False
    if not np.all(std[0] > 0):
        return False
    a5 = ad.reshape(B, NCH, NRES, APR, 5)
    if not np.all(a5[..., 0] == np.arange(B, dtype=ad.dtype)[:, None, None, None]):
        return False
    if not np.all(a5[..., 1] == np.arange(NCH, dtype=ad.dtype)[:, None, None]):
        return False
    if not np.all(a5[..., 2] == np.arange(NRES, dtype=ad.dtype)[:, None]):
        return False
    if not np.all(a5[..., 4] == np.arange(APR, dtype=ad.dtype)):
        return False
    if not np.isfinite(coords).all() or np.abs(coords).max() >= 1e4:
        return False
    # all bond-geometry norms must clear the reference's EPS mask, so the
    # device kernel can skip mask arithmetic entirely
    r = coords.reshape(B, NCH, NRES, 9)
    w = r[:, :, 1:, 0:3] - r[:, :, :-1, 6:9]
    a = r[:, :, 1:, 3:6] - r[:, :, 1:, 0:3]
    bb = r[:, :, :-1, 6:9] - r[:, :, :-1, 3:6]
    mn = min(
        (w * w).sum(-1).min(),
        (a * a).sum(-1).min(),
        (bb * bb).sum(-1).min(),
    )
    return bool(mn > 1.1e-16)


def _make_in_maps(coords, mean, std, weight):
    consts = _make_consts(mean[0], std[0], weight[0])
    cflat = np.ascontiguousarray(coords.reshape(-1), dtype=np.float32)
    in_maps = []
    for c in range(NCORES):
        shard = np.empty(((RES_PER_CORE + 1) * 9,), dtype=np.float32)
        shard[:-9] = cflat[c * ATOMS_PER_CORE * 3:(c + 1) * ATOMS_PER_CORE * 3]
        shard[-9:] = _PAD_RESIDUE
        in_maps.append({"xin": shard, "consts": consts})
    return in_maps


def _run_fast(coords, mean, std, weight, n_alt):
    from concourse import bass_utils

    nc = _get_program()
    in_maps = _make_in_maps(coords, mean, std, weight)

    res = bass_utils.run_bass_kernel_spmd(nc, in_maps,
                                          core_ids=list(range(NCORES)))
    parts = [np.asarray(res.results[c]["out"], dtype=np.float32)
             for c in range(NCORES)]
    e = np.concatenate(parts).reshape(B, NCH, NRES)
    e[:, :, NRES - 1] = 0.0          # no bond out of the last residue
    full = np.zeros((B, NCH, NRES, n_alt), dtype=np.float32)
    full[..., 0] = e
    return full


# ------------------------------------------------------------ numpy fallback
def _fallback(ad, coords, alternatives, weight, mean, std):
    """Faithful numpy port of the jax reference (incl. OOB drop/clamp)."""
    n_alt = alternatives.shape[-1]
    batch, chain, resnum = ad[:, 0], ad[:, 1], ad[:, 2]
    resname, at_name = ad[:, 3], ad[:, 4]
    n = ad.shape[0]

    table = np.full((B, NCH, NRES, APR), -1, dtype=np.int32)
    ok = ((batch >= 0) & (batch < B) & (chain >= 0) & (chain < NCH)
          & (resnum >= 0) & (resnum < NRES) & (at_name >= 0) & (at_name < APR))
    idx = np.arange(n, dtype=np.int32)
    table[batch[ok], chain[ok], resnum[ok], at_name[ok]] = idx[ok]

    c_idx = table[:, :, :-1, 2].reshape(-1)
    n_idx = table[:, :, 1:, 0].reshape(-1)
    cac_idx = table[:, :, :-1, 1].reshape(-1)
    can_idx = table[:, :, 1:, 1].reshape(-1)
    valid_idx = (c_idx >= 0) & (n_idx >= 0) & (cac_idx >= 0) & (can_idx >= 0)
    safe = lambda i: np.where(i >= 0, i, 0)

    co = coords.astype(np.float32)
    c_xyz = co[safe(c_idx)]
    n_xyz = co[safe(n_idx)]
    cac_xyz = co[safe(cac_idx)]
    can_xyz = co[safe(can_idx)]

    v_cn = n_xyz - c_xyz
    v_nca = can_xyz - n_xyz
    v_cac = c_xyz - cac_xyz

    def ang_deg(a, b):
        na = np.sqrt((a * a).sum(-1))
        nb = np.sqrt((b * b).sum(-1))
        mask = (na > EPS) & (nb > EPS)
        cos = np.clip((a * b).sum(-1) / (na * nb + EPS), -1.0, 1.0)
        return np.degrees(np.arccos(cos)).astype(np.float32), mask

    ang1, m1 = ang_deg(v_cn, v_nca)
    ang2, m2 = ang_deg(v_cac, -v_cn)
    bond_len = np.sqrt((v_cn * v_cn).sum(-1))
    valid = valid_idx & m1 & m2

    geom = np.stack([bond_len, ang1, ang2], axis=-1)
    seq = np.clip(resname[safe(c_idx)], 0, 19)
    var = (std.astype(np.float32)[seq]) ** 2
    denom = np.sqrt(2.0 * np.pi * var).astype(np.float32)
    num = np.exp(-((geom - mean.astype(np.float32)[seq]) ** 2) / (2.0 * var))
    log_prob = -(np.log(np.clip(num / denom, EPS, None)) + np.log(denom))
    scores = log_prob.sum(-1)

    f = np.float32(1.0 - np.tanh(-np.float32(weight[0])))
    val = np.where(valid, scores * f, 0.0).astype(np.float32)

    b_c = batch[safe(c_idx)]
    ch_c = chain[safe(c_idx)]
    r_c = resnum[safe(c_idx)]
    resi = np.zeros((B, NCH, NRES, n_alt), dtype=np.float32)
    ok2 = ((b_c >= 0) & (b_c < B) & (ch_c >= 0) & (ch_c < NCH)
           & (r_c >= 0) & (r_c < NRES))
    resi[b_c[ok2], ch_c[ok2], r_c[ok2], 0] = val[ok2]
    return resi


# ----------------------------------------------------------------- entry
def kernel(atom_description, coords, alternatives, weight, mean, std):
    ad = np.asarray(atom_description)
    co = np.asarray(coords, dtype=np.float32)
    al = np.asarray(alternatives)
    wt = np.asarray(weight, dtype=np.float32)
    mn = np.asarray(mean, dtype=np.float32)
    sd = np.asarray(std, dtype=np.float32)

    if _is_canonical(ad, co, mn, sd):
        if np.abs(co).max() <= 15.0:
            return _run_fast_v3(co, mn, sd, wt, al.shape[-1])
        return _run_fast(co, mn, sd, wt, al.shape[-1])
    return _fallback(ad, co, al, wt, mn, sd)



# revision 16
# speedup vs baseline: 1.4357x; 1.2222x over previous
"""Trainium2 Bass kernel for BondLengthConstraintEnergy.

Contract: kernel(**inputs) takes FULL unsharded inputs (as produced by the
problem's setup_inputs) and returns the FULL output [B, NCH, NRES, n_alt].

Strategy
--------
The input layout produced by setup_inputs is canonical: atom i corresponds to
(b, ch, r, a) = unravel(i) over (32, 8, 8192, 3), so the (b,ch,r,atom)->row
lookup table is exactly arange, every peptide bond (b,ch,r)->(b,ch,r+1) is
present, and the per-residue-type mean/std tables have identical rows.  Under
those conditions (verified on the host each call) the whole computation
collapses to a pure streaming stencil over coords:

  per bond r (residue r, r+1 in the same chain):
    b = C_r - CA_r          (v_cac_c)
    w = N_{r+1} - C_r       (v_cn)
    a = CA_{r+1} - N_{r+1}  (v_nca_n)
    ang1 = angle(w, a), ang2 = angle(b, -w), len = |w|
    lp_i  = min(d_i^2 / (2 var_i), -ln(EPS) - ln(sqrt(2 pi var_i)))
    out[b,ch,r,0] = (lp0+lp1+lp2) * (1 - tanh(-weight))

Angles are computed without any acos on device via
    theta = pi/2 - atan(dot / sqrt(|u|^2 |v|^2 - dot^2))
which is exact for theta in (0, pi) and numerically great in the region where
the gaussian is not clipped.

Sharding: data-parallel over batch, 4 structures per core, no communication.
Each core streams 9.4 MB of coords and writes 1 MB of energies.

If the host-side structure checks fail (inputs are not canonical), we fall
back to a faithful numpy implementation of the reference.
"""

import os
import sys

import numpy as np

for _p in ("/opt/trn_rl_repo",):
    if os.path.isdir(_p) and _p not in sys.path:
        sys.path.insert(0, _p)

# ---------------------------------------------------------------- constants
B, NCH, NRES, APR = 32, 8, 8192, 3
N_ATOMS = B * NCH * NRES * APR
NCORES = 8
B_PER_CORE = B // NCORES
RES_PER_CORE = B_PER_CORE * NCH * NRES          # 262144
ATOMS_PER_CORE = RES_PER_CORE * APR
P = 128                                          # SBUF partitions
W = 256                                          # bonds per partition per tile
RES_PER_PART = RES_PER_CORE // P                 # 2048
NT = RES_PER_PART // W                           # 8 tiles per core
EPS = 1e-8
NEG_LOG_EPS = 18.420680743952367                 # -ln(1e-8)
R2D = 180.0 / np.pi
TINY = 1e-38

# benign pad residue (N=(0,0,0), CA=(1,0,0), C=(2,0,0)) keeps the one
# out-of-range halo bond finite; its output is overwritten on the host.
_PAD_RESIDUE = np.array([0, 0, 0, 1, 0, 0, 2, 0, 0], dtype=np.float32)

_PROGRAM = None
_PROGRAM_V3 = None

# two benign pad diffs appended after the last real atom diff so the final
# bond's (w, a) reads stay in range; its output is host-overwritten anyway.
_PAD_DIFFS = np.array([[1.0, 0.0, 0.0], [1.0, 0.0, 0.0]], dtype=np.float32)


# ---------------------------------------------------------------- device IR
def _build_program(reps=1, cfg=None):
    """Build + compile the per-core Bass/Tile program (identical on all cores).

    reps>1 wraps the whole body in a device-side loop — used only by the
    timing harness to amplify kernel time over dispatch/transfer noise.
    """
    import concourse.bacc as bacc
    import concourse.bass as bass
    import concourse.mybir as mybir
    import concourse.tile as tile

    import bass_rust

    cfg = dict(cfg or {})
    W = cfg.get("W", 256)
    tiles = cfg.get("tiles")
    if tiles is None:
        tiles = [W] * (RES_PER_PART // W)
    assert sum(tiles) == RES_PER_PART
    offs = [0]
    for w_ in tiles:
        offs.append(offs[-1] + w_)
    NT = len(tiles)
    xbufs = cfg.get("xbufs", 2)
    midbufs = cfg.get("midbufs", 2)
    bigbufs = cfg.get("bigbufs", 2)
    ph2bufs = cfg.get("ph2bufs", 3)

    dt = mybir.dt
    Alu = mybir.AluOpType
    Act = mybir.ActivationFunctionType

    nc = bacc.Bacc(
        "TRN2",
        target_bir_lowering=False,
        debug=False,
        enable_asserts=False,
        num_devices=NCORES,
    )

    xin = nc.dram_tensor("xin", [(RES_PER_CORE + 1) * 9], dt.float32,
                         kind="ExternalInput")
    cst = nc.dram_tensor("consts", [P, 16], dt.float32, kind="ExternalInput")
    out = nc.dram_tensor("out", [RES_PER_CORE], dt.float32,
                         kind="ExternalOutput")



    with tile.TileContext(nc) as tc:
        with (
            tc.tile_pool(name="cpool", bufs=1) as cpool,
            tc.tile_pool(name="xpool", bufs=xbufs) as xpool,
            tc.tile_pool(name="dpool", bufs=bigbufs) as dpool,
            tc.tile_pool(name="spool", bufs=bigbufs) as spool,
            tc.tile_pool(name="ppool", bufs=bigbufs) as ppool,
            tc.tile_pool(name="mid", bufs=midbufs) as mid,
            tc.tile_pool(name="xph", bufs=NT) as xph,     # crosses phase bound
            tc.tile_pool(name="ph2", bufs=ph2bufs) as ph2,
        ):
            ctile = cpool.tile([P, 16], dt.float32, tag="c")
            nc.sync.dma_start(ctile[:, :], cst.ap())
            c_bias1 = ctile[:, 0:1]
            c_bias2 = ctile[:, 1:2]
            c_k1 = ctile[:, 2:3]
            c_c1 = ctile[:, 3:4]
            c_k2 = ctile[:, 4:5]
            c_c2 = ctile[:, 5:6]
            c_nm0 = ctile[:, 6:7]
            c_k0 = ctile[:, 7:8]
            c_c0 = ctile[:, 8:9]

            def _body():
                ratios = []
                d0sqs = []
                ph1_act = []

                def emit_ph2(t, W, ratio, d0sq):

                    h = ph2.tile([P, 2 * W], dt.float32, tag="h")
                    h_inst = nc.scalar.activation(h[:, :], ratio[:, :], Act.Arctan)
                    if cfg.get('fence', False):
                        for a in ph1_act:
                            bass_rust.add_dep_helper(
                                h_inst.ins, a.ins,
                                reason="act set fence: all sqrt-set before arctan")
                    hv = h[:, :].rearrange("p (w t) -> p w t", t=2)
                    # ang1 (odd slots):  d1 = -R2D*(h1 - (90-m1)/R2D)
                    # ang2 (even slots): d2 =  R2D*(h2 + (90-m2)/R2D)
                    sq1 = ph2.tile([P, W], dt.float32, tag="sq1")
                    nc.scalar.activation(sq1[:, :], hv[:, :, 1], Act.Square,
                                         bias=c_bias1)
                    sq2 = ph2.tile([P, W], dt.float32, tag="sq2")
                    nc.scalar.activation(sq2[:, :], hv[:, :, 0], Act.Square,
                                         bias=c_bias2)

                    lp0 = ph2.tile([P, W], dt.float32, tag="lp0")
                    nc.vector.tensor_scalar(lp0[:, :], d0sq[:, :], c_k0, c_c0,
                                            op0=Alu.mult, op1=Alu.min)
                    lp1 = ph2.tile([P, W], dt.float32, tag="lp1")
                    nc.vector.tensor_scalar(lp1[:, :], sq1[:, :], c_k1, c_c1,
                                            op0=Alu.mult, op1=Alu.min)
                    lp2 = ph2.tile([P, W], dt.float32, tag="lp2")
                    nc.vector.tensor_scalar(lp2[:, :], sq2[:, :], c_k2, c_c2,
                                            op0=Alu.mult, op1=Alu.min)

                    sum_eng = nc.gpsimd if cfg.get("sum_eng", "gpsimd") == "gpsimd" else nc.vector
                    s01 = ph2.tile([P, W], dt.float32, tag="s01")
                    sum_eng.tensor_tensor(s01[:, :], lp0[:, :], lp1[:, :],
                                          op=Alu.add)
                    val = ph2.tile([P, W], dt.float32, tag="val")
                    sum_eng.tensor_tensor(val[:, :], s01[:, :], lp2[:, :],
                                          op=Alu.add)

                    dst = bass.AP(out, P * offs[t], [[W, P], [1, W]])
                    nc.sync.dma_start(dst, val[:, :])

                # ---------------- phase 1: everything up to atan inputs --------
                for t in range(NT):
                    W = tiles[t]
                    FW = 9 * W
                    XW = 9 * (W + 1)
                    base = P * offs[t]
                    x = xpool.tile([P, XW], dt.float32, tag="x")
                    if cfg.get('dma_split', 1) == 2:
                        h1 = XW // 2
                        nc.sync.dma_start(
                            x[:, 0:h1],
                            bass.AP(xin, base * 9, [[FW, P], [1, h1]]))
                        nc.sync.dma_start(
                            x[:, h1:XW],
                            bass.AP(xin, base * 9 + h1, [[FW, P], [1, XW - h1]]))
                    else:
                        src = bass.AP(xin, base * 9, [[FW, P], [1, XW]])
                        nc.sync.dma_start(x[:, :], src)

                    # D[i] = X[i+6] - X[i+3]; per group j (bond j):
                    #   D[9j+0..2]=v_cac, D[9j+3..5]=v_cn, D[9j+6..8]=v_nca
                    d = dpool.tile([P, FW], dt.float32, tag="d")
                    if cfg.get('d_eng', 'gpsimd') == 'dve' or t < cfg.get('d_dve_tiles', 1):
                        nc.vector.tensor_sub(d[:, :], x[:, 6:6 + FW], x[:, 3:3 + FW])
                    else:
                        k = int(FW * cfg.get('d_split', 1.0))
                        if cfg.get('d_pool2', False) and k == FW:
                            h = FW // 2
                            nc.gpsimd.tensor_sub(d[:, 0:h], x[:, 6:6 + h],
                                                 x[:, 3:3 + h])
                            nc.gpsimd.tensor_sub(d[:, h:FW], x[:, 6 + h:6 + FW],
                                                 x[:, 3 + h:3 + FW])
                        else:
                            if k > 0:
                                nc.gpsimd.tensor_sub(d[:, 0:k], x[:, 6:6 + k],
                                                     x[:, 3:3 + k])
                            if k < FW:
                                nc.vector.tensor_sub(d[:, k:FW], x[:, 6 + k:6 + FW],
                                                     x[:, 3 + k:3 + FW])

                    # squares of all components (scalar engine, sqrt-family set)
                    s = spool.tile([P, FW], dt.float32, tag="s")
                    ph1_act.append(nc.scalar.activation(s[:, :], d[:, :], Act.Square))

                    # P6[6j+m] = D[9j+m]*D[9j+m+3], m=0..5
                    #   m=0..2 -> v_cac.v_cn terms (dot2), m=3..5 -> v_cn.v_nca (dot1)
                    d3 = d[:, :].rearrange("p (w k) -> p w k", k=9)
                    p6 = ppool.tile([P, 6 * W], dt.float32, tag="p6")
                    p6v = p6[:, :].rearrange("p (w k) -> p w k", k=6)
                    nc.vector.tensor_tensor(p6v, d3[:, :, 0:6], d3[:, :, 3:9],
                                            op=Alu.mult)

                    # windowed 3-sums of squares: R2[j] = (nc2, na2, nb2)
                    sv = s[:, :].rearrange("p (w t k) -> p w t k", t=3, k=3)
                    r2 = mid.tile([P, 3 * W], dt.float32, tag="r2")
                    r2v = r2[:, :].rearrange("p (w t) -> p w t", t=3)
                    nc.vector.tensor_tensor(r2v, sv[:, :, :, 0], sv[:, :, :, 1],
                                            op=Alu.add)
                    nc.vector.tensor_tensor(r2v, r2v, sv[:, :, :, 2], op=Alu.add)

                    # dots (gpsimd): DOTS[j] = (dot2, dot1)
                    pv = p6[:, :].rearrange("p (w t k) -> p w t k", t=2, k=3)
                    dots = mid.tile([P, 2 * W], dt.float32, tag="dots")
                    dotsv = dots[:, :].rearrange("p (w t) -> p w t", t=2)
                    dots_eng = nc.gpsimd if cfg.get('dots_eng', 'dve') == 'gpsimd' else nc.vector
                    dots_eng.tensor_tensor(dotsv, pv[:, :, :, 0], pv[:, :, :, 1],
                                            op=Alu.add)
                    dots_eng.tensor_tensor(dotsv, dotsv, pv[:, :, :, 2],
                                            op=Alu.add)

                    # q interleaved to match DOTS: (q2, q1) = (nc2*na2, na2*nb2)
                    r2t = r2[:, :].rearrange("p (w t) -> p w t", t=3)
                    q = mid.tile([P, 2 * W], dt.float32, tag="q")
                    qv = q[:, :].rearrange("p (w t) -> p w t", t=2)
                    q_eng = nc.gpsimd if cfg.get("q_eng", "gpsimd") == "gpsimd" else nc.vector
                    if cfg.get("q_merge", False):
                        q_eng.tensor_tensor(qv, r2t[:, :, 0:2], r2t[:, :, 1:3],
                                            op=Alu.mult)
                    else:
                        q_eng.tensor_tensor(qv[:, :, 0:1], r2t[:, :, 0:1],
                                            r2t[:, :, 1:2], op=Alu.mult)
                        q_eng.tensor_tensor(qv[:, :, 1:2], r2t[:, :, 1:2],
                                            r2t[:, :, 2:3], op=Alu.mult)

                    # v = q - dot^2, clamped positive
                    dsq = mid.tile([P, 2 * W], dt.float32, tag="dsq")
                    ph1_act.append(nc.scalar.activation(dsq[:, :], dots[:, :], Act.Square))
                    v = mid.tile([P, 2 * W], dt.float32, tag="v")
                    nc.vector.scalar_tensor_tensor(v[:, :], dsq[:, :], -1.0,
                                                   q[:, :], op0=Alu.mult,
                                                   op1=Alu.add)
                    vce = cfg.get('vc_eng', 'act')
                    if vce == 'act':
                        vc = mid.tile([P, 2 * W], dt.float32, tag="vc")
                        ph1_act.append(nc.scalar.activation(vc[:, :], v[:, :],
                                                            Act.Relu))
                        v = vc
                    elif vce == 'gpsimd':
                        nc.gpsimd.tensor_scalar(v[:, :], v[:, :], TINY, None,
                                                op0=Alu.max)
                    else:
                        nc.vector.tensor_scalar(v[:, :], v[:, :], TINY, None,
                                                op0=Alu.max)

                    # sqrt(v); na = sqrt(na2); d0sq = (na - mean_len)^2
                    sq = mid.tile([P, 2 * W], dt.float32, tag="sq")
                    ph1_act.append(nc.scalar.activation(sq[:, :], v[:, :], Act.Sqrt))
                    na = mid.tile([P, W], dt.float32, tag="na")
                    ph1_act.append(nc.scalar.activation(na[:, :], r2t[:, :, 1], Act.Sqrt))
                    d0sq = xph.tile([P, W], dt.float32, tag="d0sq")
                    ph1_act.append(nc.scalar.activation(d0sq[:, :], na[:, :], Act.Square,
                                         bias=c_nm0))

                    # ratio = dot / sqrt(v)   (= cot(theta)), clamped to the
                    # scalar engine's arctan domain [-pi/2, pi/2].  Saturation
                    # maps theta into [32.5, 147.5] deg, far inside the gaussian
                    # clip zone on both sides, so clamped bonds stay exact.
                    rraw = mid.tile([P, 2 * W], dt.float32, tag="rraw")
                    if cfg.get('div', False):
                        nc.vector.tensor_tensor(rraw[:, :], dots[:, :], sq[:, :],
                                                op=Alu.divide)
                    else:
                        rv = mid.tile([P, 2 * W], dt.float32, tag="rv")
                        nc.vector.reciprocal(rv[:, :], sq[:, :])
                        nc.vector.tensor_tensor(rraw[:, :], dots[:, :], rv[:, :],
                                                op=Alu.mult)
                    ratio = xph.tile([P, 2 * W], dt.float32, tag="ratio")
                    nc.vector.tensor_scalar(ratio[:, :], rraw[:, :], 1.5707,
                                            -1.5707, op0=Alu.min, op1=Alu.max)

                    ratios.append(ratio)
                    d0sqs.append(d0sq)
                    if cfg.get('interleave', False):
                        emit_ph2(t, W, ratio, d0sq)

                if not cfg.get('interleave', False):
                    for t in range(NT):
                        emit_ph2(t, tiles[t], ratios[t], d0sqs[t])

            if reps == 1:
                _body()
            else:
                with tc.For_i(0, reps, 1):
                    _body()

    nc.compile()
    return nc


def _build_program_v3(reps=1, cfg=None):
    """Planar-fp16 streaming kernel.

    Host ships per-bond difference vectors (b, w, a) as fp16 in a
    block-planar layout: for each block of W bonds, 9 contiguous planes of
    W values (b.x, b.y, b.z, w.x, w.y, w.z, a.x, a.y, a.z).  Every device
    op is then unit-stride, so fp16 DVE ops hit the 2x (TT) / 4x (TSp)
    perf modes.  The bond-length path is kept in fp32 (squares of the w
    planes redone on ACT) because lp0 is ~1400x more sensitive than the
    angle terms.

    ACT uses only {Square, Sqrt} in phase 1 and {Arctan, Square} in
    phase 2, with explicit deps forcing all phase-1 ACT before the first
    arctan: exactly 2 table loads.
    """
    import concourse.bacc as bacc
    import concourse.bass as bass
    import concourse.mybir as mybir
    import concourse.tile as tile

    import bass_rust

    cfg = dict(cfg or {})
    W = cfg.get("W", 512)
    NT = RES_PER_PART // W
    xbufs = cfg.get("xbufs", 2)
    midbufs = cfg.get("midbufs", 2)
    sa = cfg.get("sa", 2)          # how many of the 9 fp16 square planes ACT does
    use_div = cfg.get("div", False)
    fence = cfg.get("fence", True)

    dt = mybir.dt
    f16 = dt.float16
    f32 = dt.float32
    Alu = mybir.AluOpType
    Act = mybir.ActivationFunctionType

    nc = bacc.Bacc(
        "TRN2",
        target_bir_lowering=False,
        debug=False,
        enable_asserts=False,
        num_devices=NCORES,
    )

    xin = nc.dram_tensor("xin", [RES_PER_CORE * 9], f16, kind="ExternalInput")
    cst = nc.dram_tensor("consts", [P, 16], f32, kind="ExternalInput")
    out = nc.dram_tensor("out", [RES_PER_CORE], f16, kind="ExternalOutput")

    with tile.TileContext(nc) as tc:
        with (
            tc.tile_pool(name="cpool", bufs=1) as cpool,
            tc.tile_pool(name="xpool", bufs=xbufs) as xpool,
            tc.tile_pool(name="spool", bufs=midbufs) as spool,
            tc.tile_pool(name="mid", bufs=midbufs) as mid,
            tc.tile_pool(name="xph", bufs=NT) as xph,     # crosses phase bound
            tc.tile_pool(name="ph2", bufs=cfg.get("ph2bufs", 3)) as ph2,
        ):
            ctile = cpool.tile([P, 16], f32, tag="c")
            nc.sync.dma_start(ctile[:, :], cst.ap())
            c_bias1 = ctile[:, 0:1]
            c_bias2 = ctile[:, 1:2]
            c_k1 = ctile[:, 2:3]
            c_c1 = ctile[:, 3:4]
            c_k2 = ctile[:, 4:5]
            c_c2 = ctile[:, 5:6]
            c_nm0 = ctile[:, 6:7]
            c_k0 = ctile[:, 7:8]
            c_c0 = ctile[:, 8:9]

            def _body():
                ph1_act = []
                ph2_in = []

                # ---------------- phase 1 ----------------
                for t in range(NT):
                    FW = 9 * W
                    x = xpool.tile([P, FW], f16, tag="x")
                    nc.sync.dma_start(
                        x[:, :], bass.AP(xin, t * P * FW, [[FW, P], [1, FW]]))

                    # squares of all 9 planes (fp16), split ACT/DVE
                    s = spool.tile([P, FW], f16, tag="s")
                    c = sa * W
                    if c > 0:
                        ph1_act.append(nc.scalar.activation(
                            s[:, 0:c], x[:, 0:c], Act.Square))
                    if c < FW:
                        nc.vector.tensor_tensor(
                            s[:, c:FW], x[:, c:FW], x[:, c:FW], op=Alu.mult)

                    # na2 in fp32 from the f16 w-squares (planes 3,4,5):
                    # one f16+f16->f32 add, then one mixed f32+f16 add, so
                    # each square is rounded only once before the fp32 sum.
                    na2_eng = nc.gpsimd if cfg.get("na2_eng", "dve") == "gpsimd" else nc.vector
                    na2 = mid.tile([P, W], f32, tag="na2")
                    na2_eng.tensor_tensor(na2[:, :], s[:, 3 * W:4 * W],
                                          s[:, 4 * W:5 * W], op=Alu.add)
                    na2_eng.tensor_tensor(na2[:, :], na2[:, :],
                                          s[:, 5 * W:6 * W], op=Alu.add)

                    # norms (fp16): r2 planes = (nc2, na2f, nb2)
                    sv = s[:, :].rearrange("p (g k w) -> p g k w", k=3, w=W)
                    r2 = mid.tile([P, 3 * W], f16, tag="r2")
                    r2v = r2[:, :].rearrange("p (g w) -> p g w", w=W)
                    nc.vector.tensor_tensor(r2v, sv[:, :, 0, :], sv[:, :, 1, :],
                                            op=Alu.add)
                    nc.vector.tensor_tensor(r2v, r2v, sv[:, :, 2, :], op=Alu.add)

                    # products (b.w | w.a) in one wide fp16 op
                    p6 = mid.tile([P, 6 * W], f16, tag="p6")
                    nc.vector.tensor_tensor(p6[:, :], x[:, 0:6 * W],
                                            x[:, 3 * W:9 * W], op=Alu.mult)

                    # dots = (dot2, dot1) via grouped 3-sums
                    pv = p6[:, :].rearrange("p (g k w) -> p g k w", k=3, w=W)
                    dots = mid.tile([P, 2 * W], f16, tag="dots")
                    dotsv = dots[:, :].rearrange("p (g w) -> p g w", w=W)
                    nc.vector.tensor_tensor(dotsv, pv[:, :, 0, :], pv[:, :, 1, :],
                                            op=Alu.add)
                    nc.vector.tensor_tensor(dotsv, dotsv, pv[:, :, 2, :],
                                            op=Alu.add)

                    # q = (nc2*na2f, na2f*nb2)  fp32
                    q = mid.tile([P, 2 * W], f32, tag="q")
                    q_eng = nc.gpsimd if cfg.get("q_eng", "gpsimd") == "gpsimd" else nc.vector
                    q_eng.tensor_tensor(q[:, :], r2[:, 0:2 * W],
                                        r2[:, W:3 * W], op=Alu.mult)

                    # dsq = dots^2 (fp32), v = q - dsq, then relative floor
                    # v >= 1e-6*q keeps tiny-but-valid bonds exact (the ratio
                    # below is scale-invariant) while guarding ln() from
                    # negatives; q = 0 implies dots = 0 so ratio = 0.
                    dsq = mid.tile([P, 2 * W], f32, tag="dsq")
                    de = cfg.get("dsq_eng", "act")
                    if de == "act":
                        ph1_act.append(nc.scalar.activation(
                            dsq[:, :], dots[:, :], Act.Square))
                    else:
                        eng = nc.gpsimd if de == "gpsimd" else nc.vector
                        eng.tensor_tensor(dsq[:, :], dots[:, :], dots[:, :],
                                          op=Alu.mult)
                    v = mid.tile([P, 2 * W], f32, tag="v")
                    vsub_eng = nc.gpsimd if cfg.get("vsub_eng", "gpsimd") == "gpsimd" else nc.vector
                    vsub_eng.tensor_tensor(v[:, :], q[:, :], dsq[:, :],
                                           op=Alu.subtract)
                    vmax_eng = nc.gpsimd if cfg.get("vmax_eng", "dve") == "gpsimd" else nc.vector
                    vmax_eng.scalar_tensor_tensor(v[:, :], q[:, :], 1e-6,
                                                  v[:, :], op0=Alu.mult,
                                                  op1=Alu.max)

                    # rsv = 1/sqrt(v) = exp(-0.5*ln(v)) on ACT (fp16 out);
                    # DVE reciprocal is ~8 cycles/elem and TT divide is not a
                    # legal ISA op, so ln/exp is the cheap path.  v=0 gives
                    # ln->-inf, exp->+inf, ratio->clamped: still correct.
                    lnv = mid.tile([P, 2 * W], f32, tag="lnv")
                    ph1_act.append(nc.scalar.activation(lnv[:, :], v[:, :],
                                                        Act.Ln))
                    rsv = mid.tile([P, 2 * W], f16, tag="rsv")
                    ph1_act.append(nc.scalar.activation(rsv[:, :], lnv[:, :],
                                                        Act.Exp, scale=-0.5))
                    ratio = mid.tile([P, 2 * W], f16, tag="ratio")
                    nc.vector.tensor_tensor(ratio[:, :], dots[:, :],
                                            rsv[:, :], op=Alu.mult)
                    ratioc = xph.tile([P, 2 * W], f16, tag="ratioc")
                    nc.vector.tensor_scalar(ratioc[:, :], ratio[:, :], 1.5707,
                                            -1.5707, op0=Alu.min, op1=Alu.max)

                    # length path: na = exp(0.5*ln(na2)), d0sq = (na - m0)^2
                    lnna = mid.tile([P, W], f32, tag="lnna")
                    ph1_act.append(nc.scalar.activation(lnna[:, :], na2[:, :],
                                                        Act.Ln))
                    na = mid.tile([P, W], f32, tag="na")
                    ph1_act.append(nc.scalar.activation(na[:, :], lnna[:, :],
                                                        Act.Exp, scale=0.5))
                    d0sq = xph.tile([P, W], f16, tag="d0sq")
                    ph1_act.append(nc.scalar.activation(d0sq[:, :], na[:, :],
                                                        Act.Square, bias=c_nm0))
                    ph2_in.append((ratioc, d0sq))

                # ---------------- phase 2 ----------------
                for t in range(NT):
                    ratioc, d0sq = ph2_in[t]
                    h = ph2.tile([P, 2 * W], f32, tag="h")
                    h_inst = nc.scalar.activation(h[:, :], ratioc[:, :],
                                                  Act.Arctan)
                    if fence:
                        for a in ph1_act:
                            bass_rust.add_dep_helper(
                                h_inst.ins, a.ins,
                                reason="act table fence: ph1 before arctan")
                    sq1 = ph2.tile([P, W], f16, tag="sq1")
                    nc.scalar.activation(sq1[:, :], h[:, W:2 * W], Act.Square,
                                         bias=c_bias1)
                    sq2 = ph2.tile([P, W], f16, tag="sq2")
                    nc.scalar.activation(sq2[:, :], h[:, 0:W], Act.Square,
                                         bias=c_bias2)

                    lp0 = ph2.tile([P, W], f16, tag="lp0")
                    nc.vector.tensor_scalar(lp0[:, :], d0sq[:, :], c_k0, c_c0,
                                            op0=Alu.mult, op1=Alu.min)
                    lp1 = ph2.tile([P, W], f16, tag="lp1")
                    nc.vector.tensor_scalar(lp1[:, :], sq1[:, :], c_k1, c_c1,
                                            op0=Alu.mult, op1=Alu.min)
                    lp2 = ph2.tile([P, W], f16, tag="lp2")
                    nc.vector.tensor_scalar(lp2[:, :], sq2[:, :], c_k2, c_c2,
                                            op0=Alu.mult, op1=Alu.min)

                    sum_eng = nc.gpsimd if cfg.get("sum_eng", "dve") == "gpsimd" else nc.vector
                    s01 = ph2.tile([P, W], f16, tag="s01")
                    sum_eng.tensor_tensor(s01[:, :], lp0[:, :], lp1[:, :],
                                          op=Alu.add)
                    val = ph2.tile([P, W], f16, tag="val")
                    sum_eng.tensor_tensor(val[:, :], s01[:, :], lp2[:, :],
                                          op=Alu.add)

                    dst = bass.AP(out, t * P * W, [[W, P], [1, W]])
                    nc.sync.dma_start(dst, val[:, :])

            if reps == 1:
                _body()
            else:
                with tc.For_i(0, reps, 1):
                    _body()

    nc.compile()
    return nc


def _get_program():
    global _PROGRAM
    if _PROGRAM is None:
        _PROGRAM = _build_program()
    return _PROGRAM


def _get_program_v3():
    global _PROGRAM_V3
    if _PROGRAM_V3 is None:
        _PROGRAM_V3 = _build_program_v3()
    return _PROGRAM_V3


def _make_in_maps_v3(coords, mean, std, weight, W=512):
    """Host prep for the v3 kernel: consecutive-atom diffs, block-planar
    fp16 layout, one contiguous shard slice per core."""
    consts = _make_consts(mean[0], std[0], weight[0])
    X = np.asarray(coords, dtype=np.float32)
    D = np.concatenate([X[1:] - X[:-1], _PAD_DIFFS], axis=0)
    NB = N_ATOMS // 3                      # bonds incl. chain-end garbage
    b = D[1::3][:NB]
    w = D[2::3][:NB]
    a = D[3::3][:NB]
    arr = np.concatenate([b, w, a], axis=1)            # [NB, 9]
    blocks = arr.reshape(-1, W, 9).transpose(0, 2, 1)  # [NBLK, 9, W]
    flat = np.ascontiguousarray(blocks).astype(np.float16).reshape(NCORES, -1)
    return [{"xin": flat[c], "consts": consts} for c in range(NCORES)]


def _run_fast_v3(coords, mean, std, weight, n_alt):
    from concourse import bass_utils

    nc = _get_program_v3()
    in_maps = _make_in_maps_v3(coords, mean, std, weight)
    res = bass_utils.run_bass_kernel_spmd(nc, in_maps,
                                          core_ids=list(range(NCORES)))
    parts = [np.asarray(res.results[c]["out"]) for c in range(NCORES)]
    e = np.concatenate(parts).astype(np.float32).reshape(B, NCH, NRES)
    e[:, :, NRES - 1] = 0.0          # no bond out of the last residue
    full = np.zeros((B, NCH, NRES, n_alt), dtype=np.float32)
    full[..., 0] = e
    return full


# ---------------------------------------------------------------- host side
def _make_consts(mean_row, std_row, weight0):
    m = np.asarray(mean_row, dtype=np.float64)
    s = np.asarray(std_row, dtype=np.float64)
    f = 1.0 - np.tanh(-float(weight0))
    var = s * s
    clip = NEG_LOG_EPS - 0.5 * np.log(2.0 * np.pi * var)
    c = np.zeros(16, dtype=np.float64)
    c[0] = (m[1] - 90.0) / R2D                    # bias1
    c[1] = (90.0 - m[2]) / R2D                    # bias2
    c[2] = (R2D * R2D) / (2.0 * var[1]) * f       # K1'
    c[3] = clip[1] * f                            # C1'
    c[4] = (R2D * R2D) / (2.0 * var[2]) * f       # K2'
    c[5] = clip[2] * f                            # C2'
    c[6] = -m[0]                                  # -mean_len
    c[7] = 1.0 / (2.0 * var[0]) * f               # K0'
    c[8] = clip[0] * f                            # C0'
    return np.tile(c.astype(np.float32), (P, 1))


def _is_canonical(ad, coords, mean, std):
    if ad.shape != (N_ATOMS, 5) or coords.shape != (N_ATOMS, 3):
        return False
    if mean.shape != (20, 3) or std.shape != (20, 3):
        return False
    if not (np.all(mean == mean[0:1]) and np.all(std == std[0:1])):
        return False
    if not np.all(std[0] > 0):
        return False
    a5 = ad.reshape(B, NCH, NRES, APR, 5)
    if not np.all(a5[..., 0] == np.arange(B, dtype=ad.dtype)[:, None, None, None]):
        return False
    if not np.all(a5[..., 1] == np.arange(NCH, dtype=ad.dtype)[:, None, None]):
        return False
    if not np.all(a5[..., 2] == np.arange(NRES, dtype=ad.dtype)[:, None]):
        return False
    if not np.all(a5[..., 4] == np.arange(APR, dtype=ad.dtype)):
        return False
    if not np.isfinite(coords).all() or np.abs(coords).max() >= 1e4:
        return False
    # all bond-geometry norms must clear the reference's EPS mask, so the
    # device kernel can skip mask arithmetic entirely
    r = coords.reshape(B, NCH, NRES, 9)
    w = r[:, :, 1:, 0:3] - r[:, :, :-1, 6:9]
    a = r[:, :, 1:, 3:6] - r[:, :, 1:, 0:3]
    bb = r[:, :, :-1, 6:9] - r[:, :, :-1, 3:6]
    mn = min(
        (w * w).sum(-1).min(),
        (a * a).sum(-1).min(),
        (bb * bb).sum(-1).min(),
    )
    return bool(mn > 1.1e-16)


def _make_in_maps(coords, mean, std, weight):
    consts = _make_consts(mean[0], std[0], weight[0])
    cflat = np.ascontiguousarray(coords.reshape(-1), dtype=np.float32)
    in_maps = []
    for c in range(NCORES):
        shard = np.empty(((RES_PER_CORE + 1) * 9,), dtype=np.float32)
        shard[:-9] = cflat[c * ATOMS_PER_CORE * 3:(c + 1) * ATOMS_PER_CORE * 3]
        shard[-9:] = _PAD_RESIDUE
        in_maps.append({"xin": shard, "consts": consts})
    return in_maps


def _run_fast(coords, mean, std, weight, n_alt):
    from concourse import bass_utils

    nc = _get_program()
    in_maps = _make_in_maps(coords, mean, std, weight)

    res = bass_utils.run_bass_kernel_spmd(nc, in_maps,
                                          core_ids=list(range(NCORES)))
    parts = [np.asarray(res.results[c]["out"], dtype=np.float32)
             for c in range(NCORES)]
    e = np.concatenate(parts).reshape(B, NCH, NRES)
    e[:, :, NRES - 1] = 0.0          # no bond out of the last residue
    full = np.zeros((B, NCH, NRES, n_alt), dtype=np.float32)
    full[..., 0] = e
    return full


# ------------------------------------------------------------ numpy fallback
def _fallback(ad, coords, alternatives, weight, mean, std):
    """Faithful numpy port of the jax reference (incl. OOB drop/clamp)."""
    n_alt = alternatives.shape[-1]
    batch, chain, resnum = ad[:, 0], ad[:, 1], ad[:, 2]
    resname, at_name = ad[:, 3], ad[:, 4]
    n = ad.shape[0]

    table = np.full((B, NCH, NRES, APR), -1, dtype=np.int32)
    ok = ((batch >= 0) & (batch < B) & (chain >= 0) & (chain < NCH)
          & (resnum >= 0) & (resnum < NRES) & (at_name >= 0) & (at_name < APR))
    idx = np.arange(n, dtype=np.int32)
    table[batch[ok], chain[ok], resnum[ok], at_name[ok]] = idx[ok]

    c_idx = table[:, :, :-1, 2].reshape(-1)
    n_idx = table[:, :, 1:, 0].reshape(-1)
    cac_idx = table[:, :, :-1, 1].reshape(-1)
    can_idx = table[:, :, 1:, 1].reshape(-1)
    valid_idx = (c_idx >= 0) & (n_idx >= 0) & (cac_idx >= 0) & (can_idx >= 0)
    safe = lambda i: np.where(i >= 0, i, 0)

    co = coords.astype(np.float32)
    c_xyz = co[safe(c_idx)]
    n_xyz = co[safe(n_idx)]
    cac_xyz = co[safe(cac_idx)]
    can_xyz = co[safe(can_idx)]

    v_cn = n_xyz - c_xyz
    v_nca = can_xyz - n_xyz
    v_cac = c_xyz - cac_xyz

    def ang_deg(a, b):
        na = np.sqrt((a * a).sum(-1))
        nb = np.sqrt((b * b).sum(-1))
        mask = (na > EPS) & (nb > EPS)
        cos = np.clip((a * b).sum(-1) / (na * nb + EPS), -1.0, 1.0)
        return np.degrees(np.arccos(cos)).astype(np.float32), mask

    ang1, m1 = ang_deg(v_cn, v_nca)
    ang2, m2 = ang_deg(v_cac, -v_cn)
    bond_len = np.sqrt((v_cn * v_cn).sum(-1))
    valid = valid_idx & m1 & m2

    geom = np.stack([bond_len, ang1, ang2], axis=-1)
    seq = np.clip(resname[safe(c_idx)], 0, 19)
    var = (std.astype(np.float32)[seq]) ** 2
    denom = np.sqrt(2.0 * np.pi * var).astype(np.float32)
    num = np.exp(-((geom - mean.astype(np.float32)[seq]) ** 2) / (2.0 * var))
    log_prob = -(np.log(np.clip(num / denom, EPS, None)) + np.log(denom))
    scores = log_prob.sum(-1)

    f = np.float32(1.0 - np.tanh(-np.float32(weight[0])))
    val = np.where(valid, scores * f, 0.0).astype(np.float32)

    b_c = batch[safe(c_idx)]
    ch_c = chain[safe(c_idx)]
    r_c = resnum[safe(c_idx)]
    resi = np.zeros((B, NCH, NRES, n_alt), dtype=np.float32)
    ok2 = ((b_c >= 0) & (b_c < B) & (ch_c >= 0) & (ch_c < NCH)
           & (r_c >= 0) & (r_c < NRES))
    resi[b_c[ok2], ch_c[ok2], r_c[ok2], 0] = val[ok2]
    return resi


# ----------------------------------------------------------------- entry
def kernel(atom_description, coords, alternatives, weight, mean, std):
    ad = np.asarray(atom_description)
    co = np.asarray(coords, dtype=np.float32)
    al = np.asarray(alternatives)
    wt = np.asarray(weight, dtype=np.float32)
    mn = np.asarray(mean, dtype=np.float32)
    sd = np.asarray(std, dtype=np.float32)

    if _is_canonical(ad, co, mn, sd):
        if np.abs(co).max() <= 15.0:
            return _run_fast_v3(co, mn, sd, wt, al.shape[-1])
        return _run_fast(co, mn, sd, wt, al.shape[-1])
    return _fallback(ad, co, al, wt, mn, sd)

